# revision 1
# baseline (speedup 1.0000x reference)
"""Trainium2 Bass kernel for the EpistemicCuriosity module (embedding_lookup).

Data-parallel across 8 NeuronCores: the batch (65536) is split into 8 shards
of 8192 rows; the small MLP weights are replicated. Each core computes

    hidden  = relu(state @ W1_state + W1_act[action] + b1)      # [b, 256]
    pred    = hidden @ W2 + b2                                  # [b, 512]
    pe      = mean((pred - next_state)**2, axis=-1)             # [b]

for its shard, then the per-core sum of pe is AllReduced so every core can
form the updated novelty-buffer mean/std and emit

    nr      = (pe - mean_novelty) / std_novelty

on device. The novelty-buffer scalars (sum, sum-of-squares, replaced value)
are computed host-side from the replicated `novelty_history` input and passed
down as a tiny aux tensor; only pe.mean() needs cross-core communication.

Layout notes (per 512-row group, 4 subtiles of 128 rows):
 - state rows land one-per-partition; features are moved to partitions with
   4 PE transposes per subtile (PSUM), copied back to SBUF by the ACT engine.
 - matmul1 computes hiddenT (hidden units on partitions) so that
   * the W1_act[action] gather (batch-major) is folded in as a PE
     transpose accumulated into the same PSUM group,
   * b1 becomes a per-partition bias applied by the DVE relu,
   * matmul2 needs no further transposes (hiddenT is its lhsT).
 - b2 enters PSUM via a K=1 outer-product matmul with a ones row.
 - (pred - next) on DVE; square+row-sum on ACT via activation(Square,
   scale=1/sqrt(512), accum_out), giving pe directly.
"""

import sys

sys.path.insert(0, "/opt/trn_rl_repo")

from contextlib import ExitStack

import numpy as np

import concourse.bass as bass  # noqa: F401  (registers AP machinery)
import concourse.mybir as mybir
import concourse.tile as tile
from concourse import bacc
from concourse.bass import IndirectOffsetOnAxis
from concourse.bass_utils import run_bass_kernel_spmd
from concourse.masks import make_identity

P = 128
F = 512          # feature dim
H = 256          # hidden dim
V = 5000         # vocab size
HIST = 1000      # novelty history length
N_CORES = 8
B = 65536
B_LOC = B // N_CORES

_BUILD_CACHE = {}


def build_nc(b_loc=B_LOC):
    if b_loc in _BUILD_CACHE:
        return _BUILD_CACHE[b_loc]

    assert b_loc % 512 == 0
    n_groups = b_loc // 512          # 512 rows per DMA group
    ncols = b_loc // P               # pe columns (one per 128-row subtile)

    nc = bacc.Bacc("TRN2", target_bir_lowering=False, debug=False,
                   num_devices=N_CORES)
    f32 = mybir.dt.float32
    i32 = mybir.dt.int32
    Alu = mybir.AluOpType
    Act = mybir.ActivationFunctionType

    state = nc.dram_tensor("state", [b_loc, F], f32, kind="ExternalInput")
    nxt = nc.dram_tensor("next_state", [b_loc, F], f32, kind="ExternalInput")
    action = nc.dram_tensor("action", [b_loc], i32, kind="ExternalInput")
    w1s = nc.dram_tensor("w1_state", [F, H], f32, kind="ExternalInput")
    w1a = nc.dram_tensor("w1_act", [V, H], f32, kind="ExternalInput")
    b1 = nc.dram_tensor("b1", [H], f32, kind="ExternalInput")
    w2 = nc.dram_tensor("w2", [H, F], f32, kind="ExternalInput")
    b2 = nc.dram_tensor("b2", [F], f32, kind="ExternalInput")
    # aux = [S, Q - v^2, v, 0...] from the novelty history (host-computed)
    aux = nc.dram_tensor("aux", [8], f32, kind="ExternalInput")
    pe_out = nc.dram_tensor("pe_out", [b_loc], f32, kind="ExternalOutput")
    nr_out = nc.dram_tensor("nr_out", [b_loc], f32, kind="ExternalOutput")

    f32r = mybir.dt.float32r
    with tile.TileContext(nc) as tc, ExitStack() as ctx:
        const = ctx.enter_context(tc.tile_pool(name="const", bufs=1))
        sbuf = ctx.enter_context(tc.tile_pool(name="sbuf", bufs=5))
        sb2 = ctx.enter_context(tc.tile_pool(name="sb2", bufs=2))
        dram = ctx.enter_context(tc.tile_pool(name="dram", bufs=1, space="DRAM"))

        ident = const.tile([P, P], f32)
        make_identity(nc, ident[:])
        # fp32 weights staged, then rounded to f32r (single-pass PE matmuls)
        w1s_st = const.tile([P, 4, H], f32)
        nc.sync.dma_start(out=w1s_st[:], in_=w1s[:].rearrange("(k p) h -> p k h", p=P))
        w1s_r = const.tile([P, 4, H], f32r)
        nc.vector.tensor_copy(out=w1s_r[:], in_=w1s_st[:])
        w2_st = const.tile([P, 2, F], f32)
        nc.sync.dma_start(out=w2_st[:], in_=w2[:].rearrange("(j p) f -> p j f", p=P))
        w2_r = const.tile([P, 2, F], f32r)
        nc.vector.tensor_copy(out=w2_r[:], in_=w2_st[:])
        b1_sb = const.tile([P, 2], f32)
        nc.sync.dma_start(out=b1_sb[:], in_=b1[:].rearrange("(m p) -> p m", p=P))
        b2_st = const.tile([1, F], f32)
        nc.sync.dma_start(out=b2_st[:], in_=b2[:][None, :])
        b2_r = const.tile([1, F], f32r)
        nc.vector.tensor_copy(out=b2_r[:], in_=b2_st[:])
        aux_sb = const.tile([1, 8], f32)
        nc.sync.dma_start(out=aux_sb[:], in_=aux[:][None, :])
        ones_row = const.tile([1, P], f32)
        nc.vector.memset(ones_row[:], 1.0)
        ones_r = const.tile([1, P], f32r)
        nc.vector.tensor_copy(out=ones_r[:], in_=ones_row[:])
        ones_col = const.tile([P, 1], f32)
        nc.vector.memset(ones_col[:], 1.0)
        pe_all = const.tile([P, ncols], f32)

        # Warm up the collectives machinery while compute runs: a dummy
        # 32-byte AllReduce issued up-front so the real one at the tail
        # doesn't pay ncfw first-call latency.
        warm_sb = const.tile([1, 8], f32)
        nc.vector.memset(warm_sb[:], 0.0)
        warm_in = dram.tile([1, 8], f32)
        warm_out = dram.tile([8, 8], f32)
        nc.gpsimd.dma_start(out=warm_in[:], in_=warm_sb[:])
        nc.gpsimd.collective_compute(
            "AllGather", Alu.bypass,
            replica_groups=[list(range(N_CORES))],
            ins=[warm_in[0:1].opt()], outs=[warm_out.opt()])

        state_h = state[:].rearrange("(g p c) f -> g p c f", c=4, p=P)
        next_h = nxt[:].rearrange("(g p c) f -> g p c f", c=4, p=P)
        act_h = action[:].rearrange("(g p c) -> g p c", c=4, p=P)

        psum = ctx.enter_context(tc.tile_pool(name="psum", bufs=1, space="PSUM"))
        psum2 = ctx.enter_context(tc.tile_pool(name="psum2", bufs=2, space="PSUM"))
        if True:
            for g in range(n_groups):
                st_g = sbuf.tile([P, 4, F], f32, tag="st")
                nc.sync.dma_start(out=st_g[:], in_=state_h[g])
                nx_g = sbuf.tile([P, 4, F], f32, tag="nx")
                nc.scalar.dma_start(out=nx_g[:], in_=next_h[g])
                act_g = sbuf.tile([P, 4], i32, tag="act")
                nc.sync.dma_start(out=act_g[:], in_=act_h[g])
                # NOTE: multi-column offset APs mis-gather on HW (only
                # CoreSim accepts them) — one indirect DMA per 128 rows.
                emb_g = sbuf.tile([P, 4, H], f32, tag="emb")
                for c in range(4):
                    nc.gpsimd.indirect_dma_start(
                        out=emb_g[:, c, :], out_offset=None,
                        in_=w1a[:],
                        in_offset=IndirectOffsetOnAxis(ap=act_g[:, c:c + 1],
                                                       axis=0))

                # stT[k] = [128 feat, 512 batch] via 16 PE transposes (f32);
                # ACT copies round PSUM -> SBUF f32r for the matmuls.
                pstk = [psum.tile([P, F], f32, tag=f"stk{k}", name=f"pstk{k}")
                        for k in range(4)]
                for c in range(4):
                    for k in range(4):
                        nc.tensor.transpose(out=pstk[k][:, c * P:(c + 1) * P],
                                            in_=st_g[:, c, k * P:(k + 1) * P],
                                            identity=ident[:])
                stT_r = sb2.tile([P, 4, F], f32r, tag="stT")
                for k in range(4):
                    nc.scalar.copy(out=stT_r[:, k, :], in_=pstk[k][:])

                # hiddenT (pre-relu): one N=512 f32r matmul per (m, k),
                # embedding rows folded in as f32 PE transposes.
                phid = [psum.tile([P, F], f32, tag=f"phid{m}", name=f"phid{m}")
                        for m in range(2)]
                for m in range(2):
                    for k in range(4):
                        nc.tensor.matmul(out=phid[m][:],
                                         lhsT=w1s_r[:, k, m * P:(m + 1) * P],
                                         rhs=stT_r[:, k, :],
                                         start=(k == 0), stop=False)
                    for c in range(4):
                        nc.tensor.matmul(out=phid[m][:, c * P:(c + 1) * P],
                                         lhsT=emb_g[:, c, m * P:(m + 1) * P],
                                         rhs=ident[:], is_transpose=True,
                                         start=False, stop=(c == 3))

                # relu(x + b1) on DVE (b1 is per-partition here), out f32r
                hidT_r = sb2.tile([P, 2, F], f32r, tag="hidT")
                for m in range(2):
                    nc.vector.tensor_scalar(out=hidT_r[:, m, :], in0=phid[m][:],
                                            scalar1=b1_sb[:, m:m + 1],
                                            scalar2=0.0,
                                            op0=Alu.add, op1=Alu.max)

                for c in range(4):
                    # pred = hiddenT.T @ W2 + b2 (b2 via K=1 outer product)
                    p2 = psum2.tile([P, F], f32, tag="p2")
                    nc.tensor.matmul(out=p2[:], lhsT=ones_r[:], rhs=b2_r[:],
                                     start=True, stop=False)
                    for j in range(2):
                        nc.tensor.matmul(out=p2[:],
                                         lhsT=hidT_r[:, j, c * P:(c + 1) * P],
                                         rhs=w2_r[:, j, :],
                                         start=False, stop=(j == 1))

                    # pe = sum(((pred - next)/sqrt(F))^2) along the row
                    terr = sb2.tile([P, F], f32, tag="terr")
                    nc.vector.tensor_tensor(out=terr[:], in0=p2[:],
                                            in1=nx_g[:, c, :], op=Alu.subtract)
                    sq = sb2.tile([P, F], f32, tag="sq")
                    col = g * 4 + c
                    nc.scalar.activation(out=sq[:], in_=terr[:],
                                         func=Act.Square,
                                         scale=float(1.0 / np.sqrt(F)),
                                         accum_out=pe_all[:, col:col + 1])

        # prediction_error shard out (device layout [p, x]; host reorders)
        nc.sync.dma_start(out=pe_out[:].rearrange("(p x) -> p x", p=P),
                          in_=pe_all[:])

        # per-core sum of pe -> AllReduce -> global sum
        rowsum = const.tile([P, 1], f32)
        nc.vector.tensor_reduce(out=rowsum[:], in_=pe_all[:],
                                axis=mybir.AxisListType.X, op=Alu.add)
        pscal = psum.tile([P, 2], f32, tag="stk0", name="pscal")
        nc.tensor.matmul(out=pscal[0:1, 0:1], lhsT=rowsum[:], rhs=ones_col[:],
                         start=True, stop=True)
        cin_sb = const.tile([1, 8], f32)
        nc.vector.memset(cin_sb[:], 0.0)
        nc.vector.tensor_copy(out=cin_sb[:, 0:1], in_=pscal[0:1, 0:1])
        cc_in = dram.tile([1, 8], f32)
        cc_out = dram.tile([8, 8], f32)
        nc.gpsimd.dma_start(out=cc_in[:], in_=cin_sb[:])
        nc.gpsimd.collective_compute(
            "AllGather", Alu.bypass,
            replica_groups=[list(range(N_CORES))],
            ins=[cc_in[0:1].opt()], outs=[cc_out.opt()])
        parts_sb = const.tile([1, N_CORES], f32)
        nc.gpsimd.dma_start(out=parts_sb[:], in_=cc_out[:, 0][None, :])
        gsum = const.tile([1, 1], f32, tag="gsum")
        nc.vector.tensor_reduce(out=gsum[:], in_=parts_sb[:],
                                axis=mybir.AxisListType.X, op=Alu.add)

        # novelty-buffer stats from scalars (everything [1,1] on partition 0):
        #   m      = global_sum / B
        #   S'     = S - v + m            (updated buffer sum)
        #   sumsq' = (Q - v^2) + m^2      (updated buffer sum of squares)
        #   var'   = sumsq' - S'^2/HIST
        #   std    = max(sqrt(var'/(HIST-1)), 1e-4)
        #   nr     = pe * (1/std) + (-S'/HIST/std)
        S_ap = aux_sb[:, 0:1]
        Qv_ap = aux_sb[:, 1:2]
        v_ap = aux_sb[:, 2:3]
        m_t = const.tile([1, 1], f32, tag="m_t")
        nc.vector.tensor_scalar(out=m_t[:], in0=gsum[:],
                                scalar1=float(1.0 / (b_loc * N_CORES)),
                                scalar2=None, op0=Alu.mult)
        sp_t = const.tile([1, 1], f32, tag="sp_t")
        nc.vector.tensor_scalar(out=sp_t[:], in0=m_t[:], scalar1=v_ap,
                                scalar2=S_ap, op0=Alu.subtract, op1=Alu.add)
        m2_t = const.tile([1, 1], f32, tag="m2_t")
        nc.vector.tensor_tensor(out=m2_t[:], in0=m_t[:], in1=m_t[:], op=Alu.mult)
        ss_t = const.tile([1, 1], f32, tag="ss_t")
        nc.vector.tensor_scalar(out=ss_t[:], in0=m2_t[:], scalar1=Qv_ap,
                                scalar2=None, op0=Alu.add)
        sp2_t = const.tile([1, 1], f32, tag="sp2_t")
        nc.vector.tensor_tensor(out=sp2_t[:], in0=sp_t[:], in1=sp_t[:], op=Alu.mult)
        var_t = const.tile([1, 1], f32, tag="var_t")
        nc.vector.tensor_scalar(out=var_t[:], in0=sp2_t[:],
                                scalar1=float(-1.0 / HIST), scalar2=ss_t[:, 0:1],
                                op0=Alu.mult, op1=Alu.add)
        nc.vector.tensor_scalar(out=var_t[:], in0=var_t[:], scalar1=0.0,
                                scalar2=None, op0=Alu.max)
        std_t = const.tile([1, 1], f32, tag="std_t")
        nc.scalar.activation(out=std_t[:], in_=var_t[:], func=Act.Sqrt,
                             scale=float(1.0 / (HIST - 1)))
        nc.vector.tensor_scalar(out=std_t[:], in0=std_t[:], scalar1=1e-4,
                                scalar2=None, op0=Alu.max)
        inv_t = const.tile([1, 1], f32, tag="inv_t")
        nc.vector.reciprocal(out=inv_t[:], in_=std_t[:])
        bias_t = const.tile([1, 1], f32, tag="bias_t")
        nc.vector.tensor_scalar(out=bias_t[:], in0=sp_t[:], scalar1=inv_t[:, 0:1],
                                scalar2=float(-1.0 / HIST),
                                op0=Alu.mult, op1=Alu.mult)
        pair = const.tile([1, 2], f32, tag="pair")
        nc.vector.tensor_copy(out=pair[:, 0:1], in_=inv_t[:])
        nc.vector.tensor_copy(out=pair[:, 1:2], in_=bias_t[:])

        # broadcast (1/std, -mean/std) to all partitions via a K=1 matmul
        pbc = psum.tile([P, 2], f32, tag="stk1", name="pbc")
        nc.tensor.matmul(out=pbc[:], lhsT=ones_row[:], rhs=pair[:],
                         start=True, stop=True)
        bc_sb = const.tile([P, 2], f32)
        nc.vector.tensor_copy(out=bc_sb[:], in_=pbc[:])

        nr_all = const.tile([P, ncols], f32)
        nc.vector.tensor_scalar(out=nr_all[:], in0=pe_all[:],
                                scalar1=bc_sb[:, 0:1], scalar2=bc_sb[:, 1:2],
                                op0=Alu.mult, op1=Alu.add)
        nc.sync.dma_start(out=nr_out[:].rearrange("(p x) -> p x", p=P),
                          in_=nr_all[:])

    nc.compile()
    _BUILD_CACHE[b_loc] = nc
    return nc


def _make_in_maps(state, action, next_state, novelty_history, history_idx,
                  W1_state, W1_act, b1, W2, b2, b_loc=B_LOC):
    state = np.ascontiguousarray(np.asarray(state, dtype=np.float32))
    next_state = np.ascontiguousarray(np.asarray(next_state, dtype=np.float32))
    action = np.ascontiguousarray(np.asarray(action).astype(np.int32))
    w1s = np.ascontiguousarray(np.asarray(W1_state, dtype=np.float32))
    w1a = np.ascontiguousarray(np.asarray(W1_act, dtype=np.float32))
    b1 = np.ascontiguousarray(np.asarray(b1, dtype=np.float32))
    w2 = np.ascontiguousarray(np.asarray(W2, dtype=np.float32))
    b2 = np.ascontiguousarray(np.asarray(b2, dtype=np.float32))
    nh = np.asarray(novelty_history, dtype=np.float32)

    idx = int(np.asarray(history_idx)) % HIST
    v = np.float32(nh[idx])
    S = np.float32(nh.sum(dtype=np.float32))
    Q = np.float32((nh.astype(np.float32) ** 2).sum(dtype=np.float32))
    aux = np.zeros(8, dtype=np.float32)
    aux[0] = S
    aux[1] = Q - v * v
    aux[2] = v

    in_maps = []
    for i in range(N_CORES):
        sl = slice(i * b_loc, (i + 1) * b_loc)
        in_maps.append({
            "state": state[sl],
            "next_state": next_state[sl],
            "action": action[sl],
            "w1_state": w1s,
            "w1_act": w1a,
            "b1": b1,
            "w2": w2,
            "b2": b2,
            "aux": aux,
        })
    return in_maps


def _unshard(results, b_loc=B_LOC):
    ngroups = b_loc // 512
    pe_parts, nr_parts = [], []
    for r in results:
        # device layout: element [p, g*4+c] = row g*512 + p*4 + c
        pe_parts.append(np.transpose(
            r["pe_out"].reshape(P, ngroups, 4), (1, 0, 2)).ravel())
        nr_parts.append(np.transpose(
            r["nr_out"].reshape(P, ngroups, 4), (1, 0, 2)).ravel())
    return (np.ascontiguousarray(np.concatenate(pe_parts)),
            np.ascontiguousarray(np.concatenate(nr_parts)))


def kernel(state, action, next_state, novelty_history, history_idx,
           W1_state, W1_act, b1, W2, b2):
    nc = build_nc(B_LOC)
    in_maps = _make_in_maps(state, action, next_state, novelty_history,
                            history_idx, W1_state, W1_act, b1, W2, b2)
    try:
        res = run_bass_kernel_spmd(nc, in_maps, core_ids=list(range(N_CORES)))
    except Exception:
        # transient NRT device errors have been observed on a cold first
        # execute; one retry has always succeeded
        res = run_bass_kernel_spmd(nc, in_maps, core_ids=list(range(N_CORES)))
    return _unshard(res.results)


def kernel_traced(state, action, next_state, novelty_history, history_idx,
                  W1_state, W1_act, b1, W2, b2, **spmd_kwargs):
    """Like kernel() but returns (outputs, BassKernelResults) for profiling."""
    nc = build_nc(B_LOC)
    in_maps = _make_in_maps(state, action, next_state, novelty_history,
                            history_idx, W1_state, W1_act, b1, W2, b2)
    res = run_bass_kernel_spmd(nc, in_maps, core_ids=list(range(N_CORES)),
                               **spmd_kwargs)
    return _unshard(res.results), res



# revision 68
# speedup vs baseline: 1.9505x; 1.9505x over previous
"""Trainium2 Bass kernel for the EpistemicCuriosity module (embedding_lookup).

Data-parallel across 8 NeuronCores: the batch (65536) is split into 8 shards
of 8192 rows; the small MLP weights are replicated. Each core computes

    hidden  = relu(state @ W1_state + W1_act[action] + b1)      # [b, 256]
    pred    = hidden @ W2 + b2                                  # [b, 512]
    pe      = mean((pred - next_state)**2, axis=-1)             # [b]

for its shard, then the per-core sum of pe is AllGathered so every core can
form the updated novelty-buffer mean/std and emit
    nr      = (pe - mean_novelty) / std_novelty

All heavy tensors travel as bf16 and in device-friendly layouts prepared
host-side:
 - state is pre-transposed to [g][feat_part][k][batch] so matmul1 needs no
   on-chip transposes (lhsT = W1_state chunks, rhs = stateT tiles).
 - b2 is folded into next_state (next' = next - b2) so no bias matmul.
 - hidden units are split even/odd across the two M-chunks (W1_state
   columns, W1_act columns and W2 rows are permuted to match host-side).
 - the embedding gather uses dma_gather(transpose=True): rows of W1_act
   arrive already transposed ([hid_part, 2, batch]) and are accumulated
   into the matmul1 PSUM with one identity matmul per chunk. The 16
   gathers are issued up-front: SWDGE descriptor generation (~4.7us per
   512 rows on the Q7) is the serial resource the rest pipelines under.
 - relu(+b1) runs on ACT (per-partition bias); the (pred - next) subtract
   and the square+row-sum both run on DVE (scalar_tensor_tensor with
   accum_out), keeping ACT off the square path.

The novelty-buffer statistics are folded host-side into a quadratic
var(m) = c0 + c1*m + c2*m^2 in the new mean pe, so the on-device tail
after the 32-byte AllGather is ~10 small ops. A dummy AllGather at kernel
start warms the collectives channel.
"""

import sys

sys.path.insert(0, "/opt/trn_rl_repo")

from contextlib import ExitStack

import ml_dtypes
import numpy as np

import concourse.bass as bass  # noqa: F401  (registers AP machinery)
import concourse.mybir as mybir
import concourse.tile as tile
from concourse import bacc
from concourse.bass import IndirectOffsetOnAxis
from concourse.bass_utils import run_bass_kernel_spmd

P = 128
F = 512          # feature dim
H = 256          # hidden dim
V = 5000         # vocab size
HIST = 1000      # novelty history length
N_CORES = 8
B = 65536
B_LOC = B // N_CORES
NG = B_LOC // 512            # 16 groups of 512 rows
# groups 0..NHYB-1 fetch their embedding rows via library-free indirect
# DMAs (batch-major + PE transpose fold-in), the rest via transposed
# dma_gather. Measured: the all-indirect variant (NHYB=16) avoids the
# ~13us mlp-library IRAM load but its bf16->f32 casting gathers cost
# 1417ns/128 rows (vs 1083 plain), the 8 extra PE transposes/group push
# TensorE to ~78us busy, and the kernel lands ~10us SLOWER overall, so
# the transposed dma_gather stream (9.3 ns/row, emb arrives pre-
# transposed) wins despite its slow start.
NHYB = 0

BF16 = ml_dtypes.bfloat16
FP8 = ml_dtypes.float8_e4m3fn
# state/W1_state feed matmul1 as fp8 DoubleRow (2 MACs/cell/cycle); W1s and
# W1_act are scaled by 16 host-side (fp8 min-normal is 2^-6; W1 entries are
# ~1e-2) and the relu's activation scale divides the 16 back out.
W1_SCALE = 16.0

_BUILD_CACHE = {}


def build_nc(b_loc=B_LOC):
    if b_loc in _BUILD_CACHE:
        return _BUILD_CACHE[b_loc]

    assert b_loc == B_LOC
    ncols = b_loc // P               # pe columns (one per 128-row subtile)

    nc = bacc.Bacc("TRN2", target_bir_lowering=False, debug=False,
                   num_devices=N_CORES)
    f32 = mybir.dt.float32
    bf16 = mybir.dt.bfloat16
    i16 = mybir.dt.int16
    i32 = mybir.dt.int32
    Alu = mybir.AluOpType
    Act = mybir.ActivationFunctionType

    fp8 = mybir.dt.float8e4
    stateT = nc.dram_tensor("stateT", [NG, P, 2, 2, 512], fp8,
                            kind="ExternalInput")
    nxt = nc.dram_tensor("nextT", [NG, P, 4, 512], bf16, kind="ExternalInput")
    idx = nc.dram_tensor("idx", [P, NG, 32], i16, kind="ExternalInput")
    act32 = (nc.dram_tensor("act32", [P, NHYB, 4], i32, kind="ExternalInput")
             if NHYB else None)
    w1a = nc.dram_tensor("w1a", [V, H], bf16, kind="ExternalInput")
    w1s = nc.dram_tensor("w1s", [P, 2, 2, 2, P], fp8, kind="ExternalInput")
    w2 = nc.dram_tensor("w2", [P, 2, F], fp8, kind="ExternalInput")
    b1 = nc.dram_tensor("b1", [P, 2], f32, kind="ExternalInput")
    ident_in = nc.dram_tensor("ident", [P, P], bf16, kind="ExternalInput")
    # aux = [c0, c1, c2, s0, 0...]: var(m) = c0 + c1*m + c2*m^2, S' = s0 + m
    aux = nc.dram_tensor("aux", [8], f32, kind="ExternalInput")
    pe_out = nc.dram_tensor("pe_out", [b_loc], f32, kind="ExternalOutput")
    nr_out = nc.dram_tensor("nr_out", [b_loc], f32, kind="ExternalOutput")

    inv_f = float(1.0 / F)

    with tile.TileContext(nc) as tc, ExitStack() as ctx:
        const = ctx.enter_context(tc.tile_pool(name="const", bufs=1))
        sbuf = ctx.enter_context(tc.tile_pool(name="sbuf", bufs=2))
        hpool = ctx.enter_context(tc.tile_pool(name="hpool", bufs=3))
        dpool = ctx.enter_context(tc.tile_pool(name="dpool", bufs=6))
        dram = ctx.enter_context(tc.tile_pool(name="dram", bufs=1, space="DRAM"))

        # identity (bf16) for the emb-add matmuls, shipped from the host so
        # the GpSimd queue stays clear for the gathers (affine_select would
        # force extra ucode-library swaps there)
        ident16 = const.tile([P, P], bf16)
        nc.sync.dma_start(out=ident16[:], in_=ident_in[:])
        ident_f = const.tile([P, P], f32)
        nc.vector.tensor_copy(out=ident_f[:], in_=ident16[:])
        negident16 = const.tile([P, P], bf16)
        nc.vector.tensor_scalar(out=negident16[:], in0=ident16[:],
                                scalar1=-1.0, scalar2=None, op0=Alu.mult)

        # PE warmup: dense matmuls at t=0 so the HAM clock-gate releases
        # (K=8/8) before the first real group arrives.
        warmp = ctx.enter_context(tc.tile_pool(name="warmp", bufs=1,
                                               space="PSUM"))
        wm = warmp.tile([P, F], f32, tag="warm", name="wm")
        for i in range(10):
            nc.tensor.matmul(out=wm[:, 0:P], lhsT=ident16[:], rhs=ident16[:],
                             start=True, stop=True)

        # idx / act32 come in over the (otherwise idle) SWDGE queue so the
        # first gathers aren't stuck behind the megabyte streams on the
        # HWDGE queues. (An early "warmup" dma_gather to pull the
        # mlp-library IRAM load forward was tried and measured ~+13us
        # slower 3/3 times — the early IRAM traffic collides with the
        # weight/input rampup — so the first real gather pays it instead.)
        idx_sb = const.tile([P, NG, 32], i16)
        nc.gpsimd.dma_start(out=idx_sb[:], in_=idx[:])
        if NHYB:
            act_sb = const.tile([P, NHYB, 4], i32)
            nc.gpsimd.dma_start(out=act_sb[:], in_=act32[:])

        # Embedding fetch, all issued up-front; the SWDGE descriptor
        # generation (~1.1us/128 rows indirect, ~4.8us/512 rows gather,
        # serial on the Q7) is the pacing resource the rest of the kernel
        # pipelines under, so nothing else may sit ahead of it on the
        # GpSimd queue. The first NHYB groups use library-free indirect
        # DMAs (batch-major rows, folded into PSUM by PE transposes); the
        # first dma_gather then pays the mlp-library IRAM load while those
        # groups compute.
        emb_bm = [const.tile([P, 4, H], f32, name=f"emb_bm{g}")
                  for g in range(NHYB)]
        for g in range(NHYB):
            for c in range(4):
                nc.gpsimd.indirect_dma_start(
                    out=emb_bm[g][:, c, :], out_offset=None,
                    in_=w1a[:],
                    in_offset=IndirectOffsetOnAxis(ap=act_sb[:, g, c:c + 1],
                                                   axis=0))
        embT = [const.tile([P, 2, 512], bf16, name=f"embT{g}")
                for g in range(NHYB, NG)]
        nidr = nc.gpsimd.to_reg(512)
        # single_packet=True measured ~15us faster end-to-end than the
        # multi-packet mode despite exceeding the nominal 64-desc packet
        # ceiling (legal-packetization drains slower per call)
        for g in range(NHYB, NG):
            nc.gpsimd.dma_gather(
                embT[g - NHYB][:], w1a[:], idx_sb[:, g, :], 512, nidr, H,
                transpose=True)

        # Warm up the collectives machinery behind the gathers (the
        # collective_compute blocks the GpSimd queue until it completes, so
        # it must not precede them): a dummy 32-byte AllGather so the real
        # one at the tail doesn't pay ncfw first-call latency.
        warm_sb = const.tile([1, 8], f32)
        nc.vector.memset(warm_sb[:], 0.0)
        warm_in = dram.tile([1, 8], f32)
        warm_out = dram.tile([8, 8], f32)
        nc.gpsimd.dma_start(out=warm_in[:], in_=warm_sb[:])
        nc.gpsimd.collective_compute(
            "AllGather", Alu.bypass,
            replica_groups=[list(range(N_CORES))],
            ins=[warm_in[0:1].opt()], outs=[warm_out.opt()])

        w1s_sb = const.tile([P, 2, 2, 2, P], fp8)
        nc.sync.dma_start(out=w1s_sb[:], in_=w1s[:])
        w2_sb = const.tile([P, 2, F], fp8)
        nc.sync.dma_start(out=w2_sb[:], in_=w2[:])
        b1_sb = const.tile([P, 2], f32)
        nc.sync.dma_start(out=b1_sb[:], in_=b1[:])
        aux_sb = const.tile([1, 8], f32)
        nc.sync.dma_start(out=aux_sb[:], in_=aux[:][None, :])

        ones_row = const.tile([1, P], f32)
        nc.vector.memset(ones_row[:], 1.0)
        ones_col = const.tile([P, 1], f32)
        nc.vector.memset(ones_col[:], 1.0)
        pe_all = const.tile([P, ncols], f32)

        psum = ctx.enter_context(tc.tile_pool(name="psum", bufs=2,
                                              space="PSUM"))
        psum2 = ctx.enter_context(tc.tile_pool(name="psum2", bufs=3,
                                               space="PSUM"))

        for g in range(NG):
            st_g = sbuf.tile([P, 2, 2, 512], fp8, tag="st")
            nc.sync.dma_start(out=st_g[:], in_=stateT[g])
            nx_g = sbuf.tile([P, 4, 512], bf16, tag="nx")
            nc.scalar.dma_start(out=nx_g[:], in_=nxt[g])

            # hiddenT (pre-relu): 2 fp8-DoubleRow K-chunks of state @ W1s
            # (virtual K=256: feature k'*256 + 2p + j at [p, j]) with the
            # emb fold-in LAST (so the PE only waits on the gather right at
            # the end of the accumulation), even/odd hidden units in
            # separate PSUM banks.
            phid = [psum.tile([P, F], f32, tag=f"phid{i}", name=f"phid{i}")
                    for i in range(2)]
            for i in range(2):
                for k in range(2):
                    nc.tensor.matmul(out=phid[i][:],
                                     lhsT=w1s_sb[:, k, :, i, :],
                                     rhs=st_g[:, k, :, :],
                                     start=(k == 0), stop=False,
                                     perf_mode=mybir.MatmulPerfMode.DoubleRow)
                if g < NHYB:
                    for c in range(4):
                        nc.tensor.matmul(
                            out=phid[i][:, c * P:(c + 1) * P],
                            lhsT=emb_bm[g][:, c, i * P:(i + 1) * P],
                            rhs=ident_f[:], is_transpose=True,
                            start=False, stop=(c == 3))
                else:
                    nc.tensor.matmul(out=phid[i][:], lhsT=ident16[:],
                                     rhs=embT[g - NHYB][:, i, :],
                                     start=False, stop=True)

            # relu(x/16 + b1) on ACT (b1 per-partition, the 1/16 undoes the
            # host-side W1 fp8 scaling), out fp8 in DoubleRow layout
            # (partition p, slot j = hidden unit 2p+j)
            hidT = hpool.tile([P, 2, F], fp8, tag="hidT")
            for i in range(2):
                nc.scalar.activation(out=hidT[:, i, :], in_=phid[i][:],
                                     func=Act.Relu, bias=b1_sb[:, i:i + 1],
                                     scale=float(1.0 / W1_SCALE))

            for c in range(4):
                # pred - next accumulated in PSUM: one fp8-DoubleRow matmul
                # (virtual K=256 hidden units) plus a -I @ next fold-in
                p2 = psum2.tile([P, F], f32, tag="p2")
                nc.tensor.matmul(out=p2[:],
                                 lhsT=hidT[:, :, c * P:(c + 1) * P],
                                 rhs=w2_sb[:], start=True, stop=False,
                                 perf_mode=mybir.MatmulPerfMode.DoubleRow)
                nc.tensor.matmul(out=p2[:], lhsT=negident16[:],
                                 rhs=nx_g[:, c, :], start=False, stop=True)
                col = g * 4 + c
                # pe column accumulation as sum((diff/F)*diff), split
                # DVE/ACT to balance the engines (a DVE op may read only
                # one PSUM operand, so the DVE path copies to SBUF first)
                if c % 2 == 0:
                    diff = dpool.tile([P, F], bf16, tag="diff")
                    nc.vector.tensor_copy(out=diff[:], in_=p2[:])
                    sq = dpool.tile([P, F], bf16, tag="sq")
                    nc.vector.scalar_tensor_tensor(
                        out=sq[:], in0=diff[:], scalar=inv_f,
                        in1=diff[:], op0=Alu.mult, op1=Alu.mult,
                        accum_out=pe_all[:, col:col + 1])
                else:
                    sq = dpool.tile([P, F], bf16, tag="sq")
                    nc.scalar.activation(
                        out=sq[:], in_=p2[:], func=Act.Square,
                        scale=float(1.0 / np.sqrt(F)),
                        accum_out=pe_all[:, col:col + 1])

        # prediction_error shard out (device layout [p, g*4+c]; host reorders)
        nc.sync.dma_start(out=pe_out[:].rearrange("(p x) -> p x", p=P),
                          in_=pe_all[:])

        # per-core sum of pe -> AllGather -> global sum. (A two-phase
        # partial AllGather overlapping the pipeline drain was tried and
        # measured ~+6us slower: the second collective serializes behind
        # the first on the GpSimd queue/CC engine.)
        rowsum = const.tile([P, 1], f32)
        nc.vector.tensor_reduce(out=rowsum[:], in_=pe_all[:],
                                axis=mybir.AxisListType.X, op=Alu.add)
        pscal = warmp.tile([P, 2], f32, tag="warm", name="pscal")
        nc.tensor.matmul(out=pscal[0:1, 0:1], lhsT=rowsum[:], rhs=ones_col[:],
                         start=True, stop=True)
        cin_sb = const.tile([1, 8], f32)
        nc.vector.memset(cin_sb[:], 0.0)
        nc.vector.tensor_copy(out=cin_sb[:, 0:1], in_=pscal[0:1, 0:1])
        cc_in = dram.tile([1, 8], f32)
        cc_out = dram.tile([8, 8], f32)
        nc.sync.dma_start(out=cc_in[:], in_=cin_sb[:])
        nc.gpsimd.collective_compute(
            "AllGather", Alu.bypass,
            replica_groups=[list(range(N_CORES))],
            ins=[cc_in[0:1].opt()], outs=[cc_out.opt()])
        parts_sb = const.tile([1, N_CORES], f32)
        nc.scalar.dma_start(out=parts_sb[:], in_=cc_out[:, 0][None, :])
        gsum = const.tile([1, 1], f32, tag="gsum")
        nc.vector.tensor_reduce(out=gsum[:], in_=parts_sb[:],
                                axis=mybir.AxisListType.X, op=Alu.add)

        # novelty-buffer stats (everything [1,1] on partition 0):
        #   m    = global_sum / B
        #   var  = c0 + c1*m + c2*m^2      (host-folded polynomial)
        #   std  = max(sqrt(max(var,0)/(HIST-1)), 1e-4)
        #   nr   = pe * (1/std) - (s0 + m)/HIST * (1/std)
        c0_ap = aux_sb[:, 0:1]
        c1_ap = aux_sb[:, 1:2]
        c2_ap = aux_sb[:, 2:3]
        s0_ap = aux_sb[:, 3:4]
        m_t = const.tile([1, 1], f32, tag="m_t")
        nc.vector.tensor_scalar(out=m_t[:], in0=gsum[:],
                                scalar1=float(1.0 / B),
                                scalar2=None, op0=Alu.mult)
        u_t = const.tile([1, 1], f32, tag="u_t")
        nc.vector.tensor_scalar(out=u_t[:], in0=m_t[:], scalar1=c2_ap,
                                scalar2=c1_ap, op0=Alu.mult, op1=Alu.add)
        var_t = const.tile([1, 1], f32, tag="var_t")
        nc.vector.scalar_tensor_tensor(out=var_t[:], in0=u_t[:], scalar=1.0,
                                       in1=m_t[:], op0=Alu.mult, op1=Alu.mult)
        nc.vector.tensor_scalar(out=var_t[:], in0=var_t[:], scalar1=c0_ap,
                                scalar2=0.0, op0=Alu.add, op1=Alu.max)
        std_t = const.tile([1, 1], f32, tag="std_t")
        nc.scalar.activation(out=std_t[:], in_=var_t[:], func=Act.Sqrt,
                             scale=float(1.0 / (HIST - 1)))
        nc.vector.tensor_scalar(out=std_t[:], in0=std_t[:], scalar1=1e-4,
                                scalar2=None, op0=Alu.max)
        inv_t = const.tile([1, 1], f32, tag="inv_t")
        nc.vector.reciprocal(out=inv_t[:], in_=std_t[:])
        bias_t = const.tile([1, 1], f32, tag="bias_t")
        nc.vector.tensor_scalar(out=bias_t[:], in0=m_t[:], scalar1=s0_ap,
                                scalar2=float(-1.0 / HIST),
                                op0=Alu.add, op1=Alu.mult)
        nc.vector.scalar_tensor_tensor(out=bias_t[:], in0=bias_t[:],
                                       scalar=1.0, in1=inv_t[:],
                                       op0=Alu.mult, op1=Alu.mult)
        pair = const.tile([1, 2], f32, tag="pair")
        nc.vector.tensor_copy(out=pair[:, 0:1], in_=inv_t[:])
        nc.vector.tensor_copy(out=pair[:, 1:2], in_=bias_t[:])

        # broadcast (1/std, -mean/std) to all partitions via a K=1 matmul
        pbc = warmp.tile([P, 2], f32, tag="warm", name="pbc")
        nc.tensor.matmul(out=pbc[:], lhsT=ones_row[:], rhs=pair[:],
                         start=True, stop=True)
        bc_sb = const.tile([P, 2], f32)
        nc.vector.tensor_copy(out=bc_sb[:], in_=pbc[:])

        nr_all = const.tile([P, ncols], f32)
        nc.vector.tensor_scalar(out=nr_all[:], in0=pe_all[:],
                                scalar1=bc_sb[:, 0:1], scalar2=bc_sb[:, 1:2],
                                op0=Alu.mult, op1=Alu.add)
        nc.sync.dma_start(out=nr_out[:].rearrange("(p x) -> p x", p=P),
                          in_=nr_all[:])

    nc.compile()
    _BUILD_CACHE[b_loc] = nc
    return nc


def _make_in_maps(state, action, next_state, novelty_history, history_idx,
                  W1_state, W1_act, b1, W2, b2, b_loc=B_LOC):
    state = np.asarray(state, dtype=np.float32)
    next_state = np.asarray(next_state, dtype=np.float32)
    action = np.asarray(action)
    w1s = np.asarray(W1_state, dtype=np.float32)
    w1a = np.asarray(W1_act, dtype=np.float32)
    b1 = np.asarray(b1, dtype=np.float32)
    w2 = np.asarray(W2, dtype=np.float32)
    b2 = np.asarray(b2, dtype=np.float32)
    nh = np.asarray(novelty_history, dtype=np.float32)

    s8 = state.astype(FP8)
    nx16 = (next_state - b2[None, :]).astype(BF16)
    a16 = action.astype(np.int16)

    # W1_state in fp8 DoubleRow layout [p, k', j, i, h']: feature
    # k'*256 + 2p + j, hidden unit 2h' + i; scaled by 16 for fp8 range
    w1s_h = np.ascontiguousarray(
        (w1s * W1_SCALE).reshape(2, P, 2, P, 2).transpose(1, 0, 2, 4, 3)
    ).astype(FP8)
    w2_h = np.ascontiguousarray(w2.reshape(P, 2, F)).astype(FP8)
    b1_h = np.ascontiguousarray(b1.reshape(P, 2))
    # w1a columns permuted so the transposed gather lands even/odd hidden
    # units on the partitions matching w1s/w2: stored[:, i*128+p] = logical
    # [:, 2p+i]; scaled by 16 to match the fp8-scaled matmul1 PSUM
    q = np.arange(H)
    perm = 2 * (q % P) + q // P
    w1a_h = np.ascontiguousarray(w1a[:, perm] * W1_SCALE).astype(BF16)

    # novelty-buffer scalars, folded into var(m) = c0 + c1*m + c2*m^2
    idx_val = int(np.asarray(history_idx)) % HIST
    v = np.float64(nh[idx_val])
    S = np.float64(nh.sum(dtype=np.float64))
    Q = np.float64((nh.astype(np.float64) ** 2).sum())
    s0 = S - v
    aux = np.zeros(8, dtype=np.float32)
    aux[0] = Q - v * v - s0 * s0 / HIST          # c0
    aux[1] = -2.0 * s0 / HIST                    # c1
    aux[2] = 1.0 - 1.0 / HIST                    # c2
    aux[3] = s0                                  # s0

    in_maps = []
    for i in range(N_CORES):
        sl = slice(i * b_loc, (i + 1) * b_loc)
        shard = s8[sl]
        # stateT[g, p, k', j, b] = state[g*512 + b, k'*256 + 2p + j]
        stateT = np.ascontiguousarray(
            shard.reshape(NG, 512, 2, P, 2).transpose(0, 3, 2, 4, 1))
        nshard = nx16[sl]
        # nextT[g, p, c, f] = next'[g*512 + c*128 + p, f]
        nextT = np.ascontiguousarray(
            nshard.reshape(NG, 4, P, 512).transpose(0, 2, 1, 3))
        act = a16[sl]
        # idx block [16, NG, 32] with idx j of group g at [j%16, g, j//16],
        # replicated into each GPSIMD Q7 core's 16-partition slice
        blk16 = act.reshape(NG, 32, 16).transpose(2, 0, 1)
        idx_h = np.empty((P, NG, 32), dtype=np.int16)
        for k in range(8):
            idx_h[16 * k:16 * k + 16] = blk16
        # int32 offsets for the indirect-DMA groups: [p, g, c] = action of
        # batch row g*512 + c*128 + p
        act32_h = (np.ascontiguousarray(
            act[:NHYB * 512].astype(np.int32).reshape(NHYB, 4, P)
            .transpose(2, 0, 1)) if NHYB else None)
        im = {
            "stateT": stateT,
            "nextT": nextT,
            "idx": idx_h,
            "ident": np.eye(P, dtype=BF16),
            "w1a": w1a_h,
            "w1s": w1s_h,
            "w2": w2_h,
            "b1": b1_h,
            "aux": aux,
        }
        if NHYB:
            im["act32"] = act32_h
        in_maps.append(im)
    return in_maps


def _unshard(results, b_loc=B_LOC):
    pe_parts, nr_parts = [], []
    for r in results:
        # device layout: element [p, g*4+c] = row g*512 + c*128 + p
        pe_parts.append(np.transpose(
            r["pe_out"].reshape(P, NG, 4), (1, 2, 0)).ravel())
        nr_parts.append(np.transpose(
            r["nr_out"].reshape(P, NG, 4), (1, 2, 0)).ravel())
    return (np.ascontiguousarray(np.concatenate(pe_parts)),
            np.ascontiguousarray(np.concatenate(nr_parts)))


def kernel(state, action, next_state, novelty_history, history_idx,
           W1_state, W1_act, b1, W2, b2):
    nc = build_nc(B_LOC)
    in_maps = _make_in_maps(state, action, next_state, novelty_history,
                            history_idx, W1_state, W1_act, b1, W2, b2)
    try:
        res = run_bass_kernel_spmd(nc, in_maps, core_ids=list(range(N_CORES)))
    except Exception:
        # transient NRT device errors have been observed on a cold first
        # execute; one retry has always succeeded
        res = run_bass_kernel_spmd(nc, in_maps, core_ids=list(range(N_CORES)))
    return _unshard(res.results)


def kernel_traced(state, action, next_state, novelty_history, history_idx,
                  W1_state, W1_act, b1, W2, b2, **spmd_kwargs):
    """Like kernel() but returns (outputs, BassKernelResults) for profiling."""
    nc = build_nc(B_LOC)
    in_maps = _make_in_maps(state, action, next_state, novelty_history,
                            history_idx, W1_state, W1_act, b1, W2, b2)
    res = run_bass_kernel_spmd(nc, in_maps, core_ids=list(range(N_CORES)),
                               **spmd_kwargs)
    return _unshard(res.results), res
